# revision 49
# baseline (speedup 1.0000x reference)
"""Trainium2 Bass kernel for nn_MixedAttnHeadEmbed (mixed-head-config attention).

v6: host pre-rope/mix; fp8 DoubleRow QK; exp split ACT/DVE; fine-grained
head-pipelined schedule; divide-based normalize.

Math (per batch b): two attention configs share q_m/k_m/v_m [B,T,2048]:
  A: h=8  heads, d_max=256, mixing e in {1024,2048} -> d in {128,256}, w0,w1
  B: h=16 heads, d_max=128, mixing e in {1024,2048} -> d in {64,128},  w2,w3
Sharding: 8 cores = 4 batches x 2 shards; shard s owns A-heads [4s,4s+4) and
B-heads [8s,8s+8) -> output cols [1024s, 1024s+1024), written [T, 1024].

Device design notes:
 - Host precomputes roped+mixed qm/km (fp8 e4m3, q-side pre-scaled by
   1/sqrt(d_max)) and mixed V (+ones column) in bf16. Scores are tiny
   (range ~[-1, 1]) so fp8 QK costs only ~1e-3 extra error.
 - Phase 1 QK^T uses MatmulPerfMode.DoubleRow: both 128-deep d-chunks are
   contracted in ONE matmul at 0.5 cycles/col.
 - exp is the binding cost: a static balancer splits chunks between ACT
   (true exp) and DVE (Schraudolph fast-exp: int16(s*128/ln2 + 16251)
   bit-viewed as bf16).
 - Strict-upper diag-block mask: 0/1 multiply on GPSIMD post-exp.
 - Phase 2 pt-stationary bf16 with ones column; normalize = tensor_scalar
   DIVIDE by the PSUM denominator column (no reciprocal round-trips);
   B-heads accumulate onto tA via scalar_tensor_tensor divide+add.
 - Schedule: 12 heads stream through paired blocks — block i emits
   phase1(head_i) chunk-by-chunk interleaved with phase2(head_{i-1})
   qc-by-qc, so PE/ACT/DVE/GPSIMD all stay fed and dependency chains
   (mm -> exp -> mask -> phase2 -> divide) are a full block long.
"""

import math
from contextlib import ExitStack, contextmanager
from dataclasses import dataclass

import numpy as np
import ml_dtypes

import concourse.bass as bass
import concourse.mybir as mybir
import concourse.tile as tile
from concourse import bacc

F32 = mybir.dt.float32
BF = mybir.dt.bfloat16
I16 = mybir.dt.int16
FP8 = mybir.dt.float8e4
NPBF = ml_dtypes.bfloat16
NPF8 = mybir.dt.np(FP8)
NEG = -1e9
P = 128
T = 1024
TK = T // P

MAGIC_A = 128.0 / math.log(2.0)
MAGIC_B = 16251.0   # 127*128 - 5.5 (centered approx err) + 0.5 (floor->round)


@dataclass(frozen=True)
class KCfg:
    pass


FULL = KCfg()

PHASE_MARKS = []  # (start_id, end_id, label) for trace analysis

mult = mybir.AluOpType.mult
add = mybir.AluOpType.add
div = mybir.AluOpType.divide
Exp = mybir.ActivationFunctionType.Exp
DR = mybir.MatmulPerfMode.DoubleRow


def build_program(cfg: KCfg = FULL):
    nc = bacc.Bacc("TRN2", target_bir_lowering=False)

    def dram(name, shape, dt, out=False):
        return nc.declare_dram_parameter(name, list(shape), dt, isOutput=out)

    # qkA ch: qmA-i0 qmA-i1 kmA-i0 kmA-i1 (d-chunk i packed for DoubleRow)
    DqkA = dram("qkA", (4, 4, P, T), FP8)
    # qkB ch: kmB-h0 kmB-h1 qmB0-h0 qmB0-h1 qmB1-h0 qmB1-h1 (64-row halves)
    DqkB = dram("qkB", (4, 6, 64, T), FP8)
    Dvm = dram("vm", (4, TK, P, 386), BF)  # [0:256] vmA, 256 ones, [257:385] vmB, 385 ones
    Dmsk = dram("msk", (2, P, P), BF)      # ch0: strict-upper -25 tri; ch1: identity
    outQ = dram("outQ", (T, 1024), BF, out=True)
    qkA_r = [DqkA[g].rearrange("c p t -> p c t") for g in range(4)]
    qkB_r = [DqkB[g].rearrange("c p t -> p c t") for g in range(4)]
    vm_r = [Dvm[g].rearrange("c p d -> p c d") for g in range(4)]
    outr = outQ.rearrange("(c p) d -> p c d", p=P)

    with ExitStack() as ctx:
        tc = ctx.enter_context(tile.TileContext(nc))
        pers = ctx.enter_context(tc.tile_pool(name="pers", bufs=1))

        qkAp = ctx.enter_context(tc.tile_pool(name="qkA", bufs=2))
        qkBp = ctx.enter_context(tc.tile_pool(name="qkB", bufs=2))
        vmp = ctx.enter_context(tc.tile_pool(name="vm", bufs=2))
        ptp = ctx.enter_context(tc.tile_pool(name="pt", bufs=5))
        tAp = ctx.enter_context(tc.tile_pool(name="tA", bufs=2))
        outp = ctx.enter_context(tc.tile_pool(name="out", bufs=2))
        scrp = ctx.enter_context(tc.tile_pool(name="scr", bufs=3))
        spsum = ctx.enter_context(tc.tile_pool(name="sp", bufs=2, space="PSUM"))
        ypsum = ctx.enter_context(tc.tile_pool(name="yp", bufs=4, space="PSUM"))

        msk = pers.tile([P, 2, P], BF, name="msk")
        tri = msk[:, 0, :]   # 0/1 mask: tri[k,q] = 1 iff q >= k
        load = {"act": 0.0, "dve": 0.0}
        state = {}

        def balanced_exp(pt, c, q0, sT):
            sz = T - q0
            ca = 0.833 * sz + 185
            cd = 1.0417 * sz + 125
            if load["act"] + ca <= load["dve"] + cd:
                load["act"] += ca
                nc.scalar.activation(pt[:, c, q0:T], sT[:, q0:T], Exp)
            else:
                load["dve"] += cd
                nc.vector.tensor_scalar(
                    out=pt[:, c, q0:T].bitcast(I16), in0=sT[:, q0:T],
                    scalar1=MAGIC_A, scalar2=MAGIC_B, op0=mult, op1=add)

        def balanced_normA(tA, qc, y, rec):
            load["dve"] += 1.0417 * 256 + 125
            nc.vector.tensor_scalar(out=tA[:, qc, :], in0=y[:, 0:256],
                                    scalar1=rec, scalar2=None, op0=mult)

        def balanced_accB(out_sl, y, rec, tA_sl):
            load["dve"] += 1.0417 * 128 + 125
            nc.vector.scalar_tensor_tensor(out=out_sl, in0=y[:, 0:128],
                                           scalar=rec, in1=tA_sl,
                                           op0=mult, op1=add)

        def prefetch(g):
            if g >= 4 or ("qkA", g) in state:
                return
            qkA = qkAp.tile([P, 4, T], FP8, tag="qkA", name="qkA")
            nc.sync.dma_start(out=qkA, in_=qkA_r[g])
            qkB = qkBp.tile([64, 6, T], FP8, tag="qkB", name="qkB")
            nc.sync.dma_start(out=qkB, in_=qkB_r[g])
            vmt = vmp.tile([P, TK, 386], BF, tag="vm", name="vm")
            nc.sync.dma_start(out=vmt, in_=vm_r[g])
            state[("qkA", g)] = qkA
            state[("qkB", g)] = qkB
            state[("vm", g)] = vmt

        # group-0 loads: qkA first (head A0 starts), then the rest
        qkA0 = qkAp.tile([P, 4, T], FP8, tag="qkA", name="qkA0")
        nc.sync.dma_start(out=qkA0, in_=qkA_r[0])
        nc.sync.dma_start(out=msk, in_=Dmsk.rearrange("c p t -> p c t"))
        qkB0 = qkBp.tile([64, 6, T], FP8, tag="qkB", name="qkB0")
        nc.sync.dma_start(out=qkB0, in_=qkB_r[0])
        vm0 = vmp.tile([P, TK, 386], BF, tag="vm", name="vm0")
        nc.sync.dma_start(out=vm0, in_=vm_r[0])
        state[("qkA", 0)] = qkA0
        state[("qkB", 0)] = qkB0
        state[("vm", 0)] = vm0

        class Head:
            """One attention head's emission state (phase1 + phase2)."""

            def __init__(self, g, kind, hh=0):
                self.g, self.kind, self.hh = g, kind, hh
                self.label = f"g{g}.{'A' if kind == 'A' else 'B%d' % hh}"
                self.pt = None
                self.ys = {}
                self.sTs = {}

            def ensure_tiles(self):
                if self.pt is None:
                    self.pt = ptp.tile([P, TK, T], BF, tag="pt", name="pt")
                    self.rec = scrp.tile([P, TK], F32, tag="rec", name="rec")
                    if self.kind == "A":
                        self.tA = tAp.tile([P, TK, 256], BF, tag="tA",
                                           name="tA")
                        state[("tA", self.g)] = self.tA
                    else:
                        if ("o", self.g) not in state:
                            state[("o", self.g)] = outp.tile(
                                [P, TK, 256], BF, tag="outt", name="outt")
                        self.outt = state[("o", self.g)]

            def p1_mms(self, c):
                self.ensure_tiles()
                q0 = P * c
                sT = spsum.tile([P, T], F32, tag="sT", name="sT")
                self.sTs[c] = sT
                pieces = ([(q0, 512), (512, T)] if c < 4 else [(q0, T)])
                if self.kind == "A":
                    qk = state[("qkA", self.g)]
                    lhsT = qk[:, 2:4, q0:q0 + P]
                    rhs = lambda a, b: qk[:, 0:2, a:b]
                else:
                    qk = state[("qkB", self.g)]
                    lhsT = qk[:, 0:2, q0:q0 + P]
                    rhs = lambda a, b: qk[:, 2 + 2 * self.hh:4 + 2 * self.hh,
                                          a:b]
                for (a, b) in pieces:
                    nc.tensor.matmul(sT[:, a:b], lhsT, rhs(a, b),
                                     start=True, stop=True, perf_mode=DR)

            def p1_fin(self, c):
                q0 = P * c
                sT = self.sTs.pop(c)
                balanced_exp(self.pt, c, q0, sT)
                nc.gpsimd.tensor_tensor(self.pt[:, c, q0:q0 + P],
                                        self.pt[:, c, q0:q0 + P], tri, mult)

            def p2_mms(self, qc):
                vm = state[("vm", self.g)]
                y = ypsum.tile([P, 512], F32, tag="y", name="y")
                self.ys[qc] = y
                dcol = 257 if self.kind == "A" else 129
                voff = 0 if self.kind == "A" else 257
                # diag chunk first: its mask dependency is the freshest
                order = ([qc] + list(range(qc))) if qc > 0 else [0]
                for i, c in enumerate(order):
                    nc.tensor.matmul(y[:, 0:dcol],
                                     self.pt[:, c, P * qc:P * qc + P],
                                     vm[:, c, voff:voff + dcol],
                                     start=(i == 0), stop=(i == qc))

            def p2_fin(self, qc):
                y = self.ys.pop(qc)
                dcol = 257 if self.kind == "A" else 129
                nc.vector.reciprocal(self.rec[:, qc:qc + 1],
                                     y[:, dcol - 1:dcol])
                if self.kind == "A":
                    balanced_normA(self.tA, qc, y, self.rec[:, qc:qc + 1])
                else:
                    tA = state[("tA", self.g)]
                    h0 = 128 * self.hh
                    balanced_accB(self.outt[:, qc, h0:h0 + 128], y,
                                  self.rec[:, qc:qc + 1],
                                  tA[:, qc, h0:h0 + 128])
                    if self.hh == 1 and qc % 2 == 1:
                        g = self.g
                        nc.sync.dma_start(
                            out=outr[:, qc - 1:qc + 1, 256 * g:256 * g + 256],
                            in_=self.outt[:, qc - 1:qc + 1, :])

        @contextmanager
        def mark(label):
            a = nc.next_id()
            yield
            PHASE_MARKS.append((a, nc.next_id(), label))

        PHASE_MARKS.clear()

        heads = []
        for g in range(4):
            heads.append(Head(g, "A"))
            heads.append(Head(g, "B", 0))
            heads.append(Head(g, "B", 1))

        # Block-pipelined emission: block i = phase1(head_i) chunk-by-chunk
        # interleaved with phase2(head_{i-1}) qc-by-qc, offset by one slot so
        # each qc's reciprocal sits well behind its y-matmuls in the queues.
        last = len(heads) - 1
        for i in range(len(heads) + 1):
            h1 = heads[i] if i < len(heads) else None
            h2 = heads[i - 1] if i > 0 else None
            lab = f"blk{i}"
            with mark(lab):
                if h1 is not None and h1.kind == "A":
                    prefetch(h1.g + 1)
                for c in range(TK):
                    if h1 is not None:
                        h1.p1_mms(c)
                        if c > 0:
                            h1.p1_fin(c - 1)
                    if h2 is not None:
                        h2.p2_mms(c)
                        if c > 0:
                            h2.p2_fin(c - 1)
                if h1 is not None:
                    h1.p1_fin(TK - 1)
                if h2 is not None:
                    h2.p2_fin(TK - 1)

    nc.compile()
    return nc


# ---------------------------------------------------------------------------
# Host side
# ---------------------------------------------------------------------------

def _rope(x, pos):
    """HF-style RoPE applied to x [T, d] at positions pos [T]; f32."""
    d = x.shape[1]
    inv = 1.0 / (10000.0 ** (np.arange(0, d, 2, dtype=np.float32) / d))
    ang = pos.astype(np.float32)[:, None] * inv[None, :]       # [T, d/2]
    ang = np.concatenate([ang, ang], 1)
    c, s = np.cos(ang), np.sin(ang)
    rh = np.concatenate([-x[:, d // 2:], x[:, :d // 2]], 1)
    return x * c + rh * s


def make_core_inputs(q, k, v, pos, weights, s, cfg: KCfg = FULL):
    """q,k,v: [T, 2048] fp32 for one batch; returns per-core input dict."""
    w0, w1, w2, w3 = [np.float32(x) for x in weights]
    fA = np.float32(1.0 / 16.0)
    fB = np.float32(1.0 / math.sqrt(128.0))

    qkA = np.zeros((4, 4, P, T), np.float32)
    qkB = np.zeros((4, 6, 64, T), np.float32)
    vm = np.zeros((4, TK, P, 386), np.float32)
    for g in range(4):
        H = 4 * s + g
        # config A (h=8, d_max=256): e=1024 -> d=128 (w0), e=2048 -> d=256 (w1)
        qmA = w1 * _rope(q[:, 256 * H:256 * H + 256], pos)
        qmA[:, :128] += w0 * _rope(q[:, 128 * H:128 * H + 128], pos)
        kmA = w1 * _rope(k[:, 256 * H:256 * H + 256], pos)
        kmA[:, :128] += w0 * _rope(k[:, 128 * H:128 * H + 128], pos)
        qkA[g, 0] = (fA * qmA[:, :128]).T
        qkA[g, 1] = (fA * qmA[:, 128:]).T
        qkA[g, 2] = kmA[:, :128].T
        qkA[g, 3] = kmA[:, 128:].T
        # config B (h=16, d_max=128): e=1024 -> d=64 (w2), e=2048 -> d=128 (w3)
        kmB = w3 * _rope(k[:, 128 * H:128 * H + 128], pos)
        kmB[:, :64] += w2 * _rope(k[:, 64 * H:64 * H + 64], pos)
        qkB[g, 0] = kmB[:, 0:64].T
        qkB[g, 1] = kmB[:, 64:128].T
        for hh in range(2):
            Hq = 8 * s + 2 * g + hh
            qmB = w3 * _rope(q[:, 128 * Hq:128 * Hq + 128], pos)
            qmB[:, :64] += w2 * _rope(q[:, 64 * Hq:64 * Hq + 64], pos)
            qkB[g, 2 + 2 * hh] = (fB * qmB[:, 0:64]).T
            qkB[g, 3 + 2 * hh] = (fB * qmB[:, 64:128]).T
        # mixed V (+ ones columns for the softmax denominators)
        vA = w1 * v[:, 256 * H:256 * H + 256].copy()
        vA[:, :128] += w0 * v[:, 128 * H:128 * H + 128]
        vB = w3 * v[:, 128 * H:128 * H + 128].copy()
        vB[:, :64] += w2 * v[:, 64 * H:64 * H + 64]
        vm[g, :, :, 0:256] = vA.reshape(TK, P, 256)
        vm[g, :, :, 256] = 1.0
        vm[g, :, :, 257:385] = vB.reshape(TK, P, 128)
        vm[g, :, :, 385] = 1.0

    j, kk = np.mgrid[0:P, 0:P]
    tri = (kk >= j).astype(np.float32)   # tri[k,q] = 1 iff q >= k
    msk = np.stack([tri, np.eye(P, dtype=np.float32)])

    return {"qkA": np.ascontiguousarray(qkA, dtype=NPF8),
            "qkB": np.ascontiguousarray(qkB, dtype=NPF8),
            "vm": np.ascontiguousarray(vm, dtype=NPBF),
            "msk": np.ascontiguousarray(msk, dtype=NPBF)}


_PROGRAM_CACHE = {}
TRACE = False
LAST_RESULT = None


def kernel(q_m, k_m, v_m, weights, attention_mask, position_ids):
    global LAST_RESULT
    from concourse.bass_utils import run_bass_kernel_spmd

    cfg = FULL
    q_m = np.asarray(q_m, np.float32)
    k_m = np.asarray(k_m, np.float32)
    v_m = np.asarray(v_m, np.float32)
    weights = np.asarray(weights, np.float32)
    attention_mask = np.asarray(attention_mask, np.float32)
    position_ids = np.asarray(position_ids)
    B, Tq, H = q_m.shape

    causal = np.where(np.tril(np.ones((Tq, Tq), bool)), 0.0, NEG).astype(np.float32)
    for b in range(B):
        assert np.array_equal(attention_mask[b, 0], causal), "non-causal mask"

    if "nc" not in _PROGRAM_CACHE:
        _PROGRAM_CACHE["nc"] = build_program(cfg)
    nc = _PROGRAM_CACHE["nc"]

    in_maps = []
    for b in range(B):
        for s in range(2):
            in_maps.append(make_core_inputs(
                q_m[b], k_m[b], v_m[b], position_ids[b], weights, s, cfg))
    res = run_bass_kernel_spmd(nc, in_maps, list(range(8)), trace=TRACE)
    LAST_RESULT = res
    out = np.zeros((B, Tq, H), np.float32)
    for b in range(B):
        for s in range(2):
            out[b, :, 1024 * s:1024 * s + 1024] = \
                res.results[2 * b + s]["outQ"].astype(np.float32)
    return out


# revision 51
# speedup vs baseline: 1.0009x; 1.0009x over previous
"""Trainium2 Bass kernel for nn_MixedAttnHeadEmbed (mixed-head-config attention).

v6: host pre-rope/mix; fp8 DoubleRow QK; exp split ACT/DVE; fine-grained
head-pipelined schedule; divide-based normalize.

Math (per batch b): two attention configs share q_m/k_m/v_m [B,T,2048]:
  A: h=8  heads, d_max=256, mixing e in {1024,2048} -> d in {128,256}, w0,w1
  B: h=16 heads, d_max=128, mixing e in {1024,2048} -> d in {64,128},  w2,w3
Sharding: 8 cores = 4 batches x 2 shards; shard s owns A-heads [4s,4s+4) and
B-heads [8s,8s+8) -> output cols [1024s, 1024s+1024), written [T, 1024].

Device design notes:
 - Host precomputes roped+mixed qm/km (fp8 e4m3, q-side pre-scaled by
   1/sqrt(d_max)) and mixed V (+ones column) in bf16. Scores are tiny
   (range ~[-1, 1]) so fp8 QK costs only ~1e-3 extra error.
 - Phase 1 QK^T uses MatmulPerfMode.DoubleRow: both 128-deep d-chunks are
   contracted in ONE matmul at 0.5 cycles/col.
 - exp is the binding cost: a static balancer splits chunks between ACT
   (true exp) and DVE (Schraudolph fast-exp: int16(s*128/ln2 + 16251)
   bit-viewed as bf16).
 - Strict-upper diag-block mask: 0/1 multiply on GPSIMD post-exp.
 - Phase 2 pt-stationary bf16 with ones column; normalize = tensor_scalar
   DIVIDE by the PSUM denominator column (no reciprocal round-trips);
   B-heads accumulate onto tA via scalar_tensor_tensor divide+add.
 - Schedule: 12 heads stream through paired blocks — block i emits
   phase1(head_i) chunk-by-chunk interleaved with phase2(head_{i-1})
   qc-by-qc, so PE/ACT/DVE/GPSIMD all stay fed and dependency chains
   (mm -> exp -> mask -> phase2 -> divide) are a full block long.
"""

import math
from contextlib import ExitStack, contextmanager
from dataclasses import dataclass

import numpy as np
import ml_dtypes

import concourse.bass as bass
import concourse.mybir as mybir
import concourse.tile as tile
from concourse import bacc

F32 = mybir.dt.float32
BF = mybir.dt.bfloat16
I16 = mybir.dt.int16
FP8 = mybir.dt.float8e4
NPBF = ml_dtypes.bfloat16
NPF8 = mybir.dt.np(FP8)
NEG = -1e9
P = 128
T = 1024
TK = T // P

MAGIC_A = 128.0 / math.log(2.0)
MAGIC_B = 16251.0   # 127*128 - 5.5 (centered approx err) + 0.5 (floor->round)


@dataclass(frozen=True)
class KCfg:
    pass


FULL = KCfg()

PHASE_MARKS = []  # (start_id, end_id, label) for trace analysis

mult = mybir.AluOpType.mult
add = mybir.AluOpType.add
div = mybir.AluOpType.divide
Exp = mybir.ActivationFunctionType.Exp
DR = mybir.MatmulPerfMode.DoubleRow


def build_program(cfg: KCfg = FULL):
    nc = bacc.Bacc("TRN2", target_bir_lowering=False)

    def dram(name, shape, dt, out=False):
        return nc.declare_dram_parameter(name, list(shape), dt, isOutput=out)

    # qkA ch: qmA-i0 qmA-i1 kmA-i0 kmA-i1 (d-chunk i packed for DoubleRow)
    DqkA = dram("qkA", (4, 4, P, T), FP8)
    # qkB ch: kmB-h0 kmB-h1 qmB0-h0 qmB0-h1 qmB1-h0 qmB1-h1 (64-row halves)
    DqkB = dram("qkB", (4, 6, 64, T), FP8)
    Dvm = dram("vm", (4, TK, P, 386), BF)  # [0:256] vmA, 256 ones, [257:385] vmB, 385 ones
    Dmsk = dram("msk", (2, P, P), BF)      # ch0: strict-upper -25 tri; ch1: identity
    outQ = dram("outQ", (T, 1024), BF, out=True)
    qkA_r = [DqkA[g].rearrange("c p t -> p c t") for g in range(4)]
    qkB_r = [DqkB[g].rearrange("c p t -> p c t") for g in range(4)]
    vm_r = [Dvm[g].rearrange("c p d -> p c d") for g in range(4)]
    outr = outQ.rearrange("(c p) d -> p c d", p=P)

    with ExitStack() as ctx:
        tc = ctx.enter_context(tile.TileContext(nc))
        pers = ctx.enter_context(tc.tile_pool(name="pers", bufs=1))

        qkAp = ctx.enter_context(tc.tile_pool(name="qkA", bufs=2))
        qkBp = ctx.enter_context(tc.tile_pool(name="qkB", bufs=2))
        vmp = ctx.enter_context(tc.tile_pool(name="vm", bufs=2))
        ptp = ctx.enter_context(tc.tile_pool(name="pt", bufs=5))
        tAp = ctx.enter_context(tc.tile_pool(name="tA", bufs=2))
        outp = ctx.enter_context(tc.tile_pool(name="out", bufs=2))
        scrp = ctx.enter_context(tc.tile_pool(name="scr", bufs=3))
        spsum = ctx.enter_context(tc.tile_pool(name="sp", bufs=2, space="PSUM"))
        ypsum = ctx.enter_context(tc.tile_pool(name="yp", bufs=4, space="PSUM"))

        msk = pers.tile([P, 2, P], BF, name="msk")
        tri = msk[:, 0, :]   # 0/1 mask: tri[k,q] = 1 iff q >= k
        load = {"act": 0.0, "dve": 0.0}
        state = {}

        def balanced_exp(pt, c, q0, sT):
            sz = T - q0
            ca = 0.833 * sz + 185
            cd = 1.0417 * sz + 125
            if load["act"] + ca <= load["dve"] + cd:
                load["act"] += ca
                nc.scalar.activation(pt[:, c, q0:T], sT[:, q0:T], Exp)
            else:
                load["dve"] += cd
                nc.vector.tensor_scalar(
                    out=pt[:, c, q0:T].bitcast(I16), in0=sT[:, q0:T],
                    scalar1=MAGIC_A, scalar2=MAGIC_B, op0=mult, op1=add)

        def balanced_normA(tA, qc, y, rec):
            load["dve"] += 1.0417 * 256 + 125
            nc.vector.tensor_scalar(out=tA[:, qc, :], in0=y[:, 0:256],
                                    scalar1=rec, scalar2=None, op0=mult)

        def balanced_accB(out_sl, y, rec, tA_sl):
            load["dve"] += 1.0417 * 128 + 125
            nc.vector.scalar_tensor_tensor(out=out_sl, in0=y[:, 0:128],
                                           scalar=rec, in1=tA_sl,
                                           op0=mult, op1=add)

        def prefetch(g):
            if g >= 4 or ("qkA", g) in state:
                return
            qkA = qkAp.tile([P, 4, T], FP8, tag="qkA", name="qkA")
            nc.sync.dma_start(out=qkA, in_=qkA_r[g])
            qkB = qkBp.tile([64, 6, T], FP8, tag="qkB", name="qkB")
            nc.sync.dma_start(out=qkB, in_=qkB_r[g])
            vmt = vmp.tile([P, TK, 386], BF, tag="vm", name="vm")
            nc.sync.dma_start(out=vmt, in_=vm_r[g])
            state[("qkA", g)] = qkA
            state[("qkB", g)] = qkB
            state[("vm", g)] = vmt

        # group-0 loads: qkA first (head A0 starts), then the rest
        qkA0 = qkAp.tile([P, 4, T], FP8, tag="qkA", name="qkA0")
        nc.sync.dma_start(out=qkA0, in_=qkA_r[0])
        nc.sync.dma_start(out=msk, in_=Dmsk.rearrange("c p t -> p c t"))
        qkB0 = qkBp.tile([64, 6, T], FP8, tag="qkB", name="qkB0")
        nc.sync.dma_start(out=qkB0, in_=qkB_r[0])
        vm0 = vmp.tile([P, TK, 386], BF, tag="vm", name="vm0")
        nc.sync.dma_start(out=vm0, in_=vm_r[0])
        state[("qkA", 0)] = qkA0
        state[("qkB", 0)] = qkB0
        state[("vm", 0)] = vm0

        class Head:
            """One attention head's emission state (phase1 + phase2)."""

            def __init__(self, g, kind, hh=0):
                self.g, self.kind, self.hh = g, kind, hh
                self.label = f"g{g}.{'A' if kind == 'A' else 'B%d' % hh}"
                self.pt = None
                self.ys = {}
                self.sTs = {}

            def ensure_tiles(self):
                if self.pt is None:
                    self.pt = ptp.tile([P, TK, T], BF, tag="pt", name="pt")
                    self.rec = scrp.tile([P, TK], F32, tag="rec", name="rec")
                    if self.kind == "A":
                        self.tA = tAp.tile([P, TK, 256], BF, tag="tA",
                                           name="tA")
                        state[("tA", self.g)] = self.tA
                    else:
                        if ("o", self.g) not in state:
                            state[("o", self.g)] = outp.tile(
                                [P, TK, 256], BF, tag="outt", name="outt")
                        self.outt = state[("o", self.g)]

            def p1_mms(self, c):
                self.ensure_tiles()
                q0 = P * c
                sT = spsum.tile([P, T], F32, tag="sT", name="sT")
                self.sTs[c] = sT
                pieces = ([(q0, 512), (512, T)] if c < 4 else [(q0, T)])
                if self.kind == "A":
                    qk = state[("qkA", self.g)]
                    lhsT = qk[:, 2:4, q0:q0 + P]
                    rhs = lambda a, b: qk[:, 0:2, a:b]
                else:
                    qk = state[("qkB", self.g)]
                    lhsT = qk[:, 0:2, q0:q0 + P]
                    rhs = lambda a, b: qk[:, 2 + 2 * self.hh:4 + 2 * self.hh,
                                          a:b]
                for (a, b) in pieces:
                    nc.tensor.matmul(sT[:, a:b], lhsT, rhs(a, b),
                                     start=True, stop=True, perf_mode=DR)

            def p1_fin(self, c):
                q0 = P * c
                sT = self.sTs.pop(c)
                balanced_exp(self.pt, c, q0, sT)
                nc.gpsimd.tensor_tensor(self.pt[:, c, q0:q0 + P],
                                        self.pt[:, c, q0:q0 + P], tri, mult)

            def p2_mms(self, qc):
                vm = state[("vm", self.g)]
                y = ypsum.tile([P, 512], F32, tag="y", name="y")
                self.ys[qc] = y
                dcol = 257 if self.kind == "A" else 129
                voff = 0 if self.kind == "A" else 257
                # diag chunk first: its mask dependency is the freshest
                order = ([qc] + list(range(qc))) if qc > 0 else [0]
                for i, c in enumerate(order):
                    nc.tensor.matmul(y[:, 0:dcol],
                                     self.pt[:, c, P * qc:P * qc + P],
                                     vm[:, c, voff:voff + dcol],
                                     start=(i == 0), stop=(i == qc))

            def p2_fin(self, qc):
                y = self.ys.pop(qc)
                dcol = 257 if self.kind == "A" else 129
                nc.vector.reciprocal(self.rec[:, qc:qc + 1],
                                     y[:, dcol - 1:dcol])
                if self.kind == "A":
                    balanced_normA(self.tA, qc, y, self.rec[:, qc:qc + 1])
                else:
                    tA = state[("tA", self.g)]
                    h0 = 128 * self.hh
                    balanced_accB(self.outt[:, qc, h0:h0 + 128], y,
                                  self.rec[:, qc:qc + 1],
                                  tA[:, qc, h0:h0 + 128])
                    if self.hh == 1 and qc % 2 == 1:
                        g = self.g
                        nc.sync.dma_start(
                            out=outr[:, qc - 1:qc + 1, 256 * g:256 * g + 256],
                            in_=self.outt[:, qc - 1:qc + 1, :])

        @contextmanager
        def mark(label):
            a = nc.next_id()
            yield
            PHASE_MARKS.append((a, nc.next_id(), label))

        PHASE_MARKS.clear()

        heads = []
        for g in range(4):
            heads.append(Head(g, "A"))
            heads.append(Head(g, "B", 0))
            heads.append(Head(g, "B", 1))

        # Block-pipelined emission: block i = phase1(head_i) chunk-by-chunk
        # interleaved with phase2(head_{i-1}) qc-by-qc, offset by one slot so
        # each qc's reciprocal sits well behind its y-matmuls in the queues.
        last = len(heads) - 1
        for i in range(len(heads) + 1):
            h1 = heads[i] if i < len(heads) else None
            h2 = heads[i - 1] if i > 0 else None
            lab = f"blk{i}"
            with mark(lab):
                if h1 is not None and h1.kind == "A":
                    prefetch(h1.g + 1)
                for c in range(TK):
                    if h1 is not None:
                        h1.p1_mms(c)
                        h1.p1_fin(c)
                    if h2 is not None:
                        h2.p2_mms(c)
                        if c > 1:
                            h2.p2_fin(c - 2)
                if h2 is not None:
                    h2.p2_fin(TK - 2)
                    h2.p2_fin(TK - 1)

    nc.compile()
    return nc


# ---------------------------------------------------------------------------
# Host side
# ---------------------------------------------------------------------------

def _rope(x, pos):
    """HF-style RoPE applied to x [T, d] at positions pos [T]; f32."""
    d = x.shape[1]
    inv = 1.0 / (10000.0 ** (np.arange(0, d, 2, dtype=np.float32) / d))
    ang = pos.astype(np.float32)[:, None] * inv[None, :]       # [T, d/2]
    ang = np.concatenate([ang, ang], 1)
    c, s = np.cos(ang), np.sin(ang)
    rh = np.concatenate([-x[:, d // 2:], x[:, :d // 2]], 1)
    return x * c + rh * s


def make_core_inputs(q, k, v, pos, weights, s, cfg: KCfg = FULL):
    """q,k,v: [T, 2048] fp32 for one batch; returns per-core input dict."""
    w0, w1, w2, w3 = [np.float32(x) for x in weights]
    fA = np.float32(1.0 / 16.0)
    fB = np.float32(1.0 / math.sqrt(128.0))

    qkA = np.zeros((4, 4, P, T), np.float32)
    qkB = np.zeros((4, 6, 64, T), np.float32)
    vm = np.zeros((4, TK, P, 386), np.float32)
    for g in range(4):
        H = 4 * s + g
        # config A (h=8, d_max=256): e=1024 -> d=128 (w0), e=2048 -> d=256 (w1)
        qmA = w1 * _rope(q[:, 256 * H:256 * H + 256], pos)
        qmA[:, :128] += w0 * _rope(q[:, 128 * H:128 * H + 128], pos)
        kmA = w1 * _rope(k[:, 256 * H:256 * H + 256], pos)
        kmA[:, :128] += w0 * _rope(k[:, 128 * H:128 * H + 128], pos)
        qkA[g, 0] = (fA * qmA[:, :128]).T
        qkA[g, 1] = (fA * qmA[:, 128:]).T
        qkA[g, 2] = kmA[:, :128].T
        qkA[g, 3] = kmA[:, 128:].T
        # config B (h=16, d_max=128): e=1024 -> d=64 (w2), e=2048 -> d=128 (w3)
        kmB = w3 * _rope(k[:, 128 * H:128 * H + 128], pos)
        kmB[:, :64] += w2 * _rope(k[:, 64 * H:64 * H + 64], pos)
        qkB[g, 0] = kmB[:, 0:64].T
        qkB[g, 1] = kmB[:, 64:128].T
        for hh in range(2):
            Hq = 8 * s + 2 * g + hh
            qmB = w3 * _rope(q[:, 128 * Hq:128 * Hq + 128], pos)
            qmB[:, :64] += w2 * _rope(q[:, 64 * Hq:64 * Hq + 64], pos)
            qkB[g, 2 + 2 * hh] = (fB * qmB[:, 0:64]).T
            qkB[g, 3 + 2 * hh] = (fB * qmB[:, 64:128]).T
        # mixed V (+ ones columns for the softmax denominators)
        vA = w1 * v[:, 256 * H:256 * H + 256].copy()
        vA[:, :128] += w0 * v[:, 128 * H:128 * H + 128]
        vB = w3 * v[:, 128 * H:128 * H + 128].copy()
        vB[:, :64] += w2 * v[:, 64 * H:64 * H + 64]
        vm[g, :, :, 0:256] = vA.reshape(TK, P, 256)
        vm[g, :, :, 256] = 1.0
        vm[g, :, :, 257:385] = vB.reshape(TK, P, 128)
        vm[g, :, :, 385] = 1.0

    j, kk = np.mgrid[0:P, 0:P]
    tri = (kk >= j).astype(np.float32)   # tri[k,q] = 1 iff q >= k
    msk = np.stack([tri, np.eye(P, dtype=np.float32)])

    return {"qkA": np.ascontiguousarray(qkA, dtype=NPF8),
            "qkB": np.ascontiguousarray(qkB, dtype=NPF8),
            "vm": np.ascontiguousarray(vm, dtype=NPBF),
            "msk": np.ascontiguousarray(msk, dtype=NPBF)}


_PROGRAM_CACHE = {}
TRACE = False
LAST_RESULT = None


def kernel(q_m, k_m, v_m, weights, attention_mask, position_ids):
    global LAST_RESULT
    from concourse.bass_utils import run_bass_kernel_spmd

    cfg = FULL
    q_m = np.asarray(q_m, np.float32)
    k_m = np.asarray(k_m, np.float32)
    v_m = np.asarray(v_m, np.float32)
    weights = np.asarray(weights, np.float32)
    attention_mask = np.asarray(attention_mask, np.float32)
    position_ids = np.asarray(position_ids)
    B, Tq, H = q_m.shape

    causal = np.where(np.tril(np.ones((Tq, Tq), bool)), 0.0, NEG).astype(np.float32)
    for b in range(B):
        assert np.array_equal(attention_mask[b, 0], causal), "non-causal mask"

    if "nc" not in _PROGRAM_CACHE:
        _PROGRAM_CACHE["nc"] = build_program(cfg)
    nc = _PROGRAM_CACHE["nc"]

    in_maps = []
    for b in range(B):
        for s in range(2):
            in_maps.append(make_core_inputs(
                q_m[b], k_m[b], v_m[b], position_ids[b], weights, s, cfg))
    res = run_bass_kernel_spmd(nc, in_maps, list(range(8)), trace=TRACE)
    LAST_RESULT = res
    out = np.zeros((B, Tq, H), np.float32)
    for b in range(B):
        for s in range(2):
            out[b, :, 1024 * s:1024 * s + 1024] = \
                res.results[2 * b + s]["outQ"].astype(np.float32)
    return out


# revision 52
# speedup vs baseline: 1.0058x; 1.0048x over previous
"""Trainium2 Bass kernel for nn_MixedAttnHeadEmbed (mixed-head-config attention).

v6: host pre-rope/mix; fp8 DoubleRow QK; exp split ACT/DVE; fine-grained
head-pipelined schedule; divide-based normalize.

Math (per batch b): two attention configs share q_m/k_m/v_m [B,T,2048]:
  A: h=8  heads, d_max=256, mixing e in {1024,2048} -> d in {128,256}, w0,w1
  B: h=16 heads, d_max=128, mixing e in {1024,2048} -> d in {64,128},  w2,w3
Sharding: 8 cores = 4 batches x 2 shards; shard s owns A-heads [4s,4s+4) and
B-heads [8s,8s+8) -> output cols [1024s, 1024s+1024), written [T, 1024].

Device design notes:
 - Host precomputes roped+mixed qm/km (fp8 e4m3, q-side pre-scaled by
   1/sqrt(d_max)) and mixed V (+ones column) in bf16. Scores are tiny
   (range ~[-1, 1]) so fp8 QK costs only ~1e-3 extra error.
 - Phase 1 QK^T uses MatmulPerfMode.DoubleRow: both 128-deep d-chunks are
   contracted in ONE matmul at 0.5 cycles/col.
 - exp is the binding cost: a static balancer splits chunks between ACT
   (true exp) and DVE (Schraudolph fast-exp: int16(s*128/ln2 + 16251)
   bit-viewed as bf16).
 - Strict-upper diag-block mask: 0/1 multiply on GPSIMD post-exp.
 - Phase 2 pt-stationary bf16 with ones column; normalize = tensor_scalar
   DIVIDE by the PSUM denominator column (no reciprocal round-trips);
   B-heads accumulate onto tA via scalar_tensor_tensor divide+add.
 - Schedule: 12 heads stream through paired blocks — block i emits
   phase1(head_i) chunk-by-chunk interleaved with phase2(head_{i-1})
   qc-by-qc, so PE/ACT/DVE/GPSIMD all stay fed and dependency chains
   (mm -> exp -> mask -> phase2 -> divide) are a full block long.
"""

import math
from contextlib import ExitStack, contextmanager
from dataclasses import dataclass

import numpy as np
import ml_dtypes

import concourse.bass as bass
import concourse.mybir as mybir
import concourse.tile as tile
from concourse import bacc

F32 = mybir.dt.float32
BF = mybir.dt.bfloat16
I16 = mybir.dt.int16
FP8 = mybir.dt.float8e4
NPBF = ml_dtypes.bfloat16
NPF8 = mybir.dt.np(FP8)
NEG = -1e9
P = 128
T = 1024
TK = T // P

MAGIC_A = 128.0 / math.log(2.0)
MAGIC_B = 16251.0   # 127*128 - 5.5 (centered approx err) + 0.5 (floor->round)


@dataclass(frozen=True)
class KCfg:
    pass


FULL = KCfg()

PHASE_MARKS = []  # (start_id, end_id, label) for trace analysis

mult = mybir.AluOpType.mult
add = mybir.AluOpType.add
div = mybir.AluOpType.divide
Exp = mybir.ActivationFunctionType.Exp
DR = mybir.MatmulPerfMode.DoubleRow


def build_program(cfg: KCfg = FULL):
    nc = bacc.Bacc("TRN2", target_bir_lowering=False)

    def dram(name, shape, dt, out=False):
        return nc.declare_dram_parameter(name, list(shape), dt, isOutput=out)

    # qkA ch: qmA-i0 qmA-i1 kmA-i0 kmA-i1 (d-chunk i packed for DoubleRow)
    DqkA = dram("qkA", (4, 4, P, T), FP8)
    # qkB ch: kmB-h0 kmB-h1 qmB0-h0 qmB0-h1 qmB1-h0 qmB1-h1 (64-row halves)
    DqkB = dram("qkB", (4, 6, 64, T), FP8)
    Dvm = dram("vm", (4, TK, P, 386), BF)  # [0:256] vmA, 256 ones, [257:385] vmB, 385 ones
    Dmsk = dram("msk", (2, P, P), BF)      # ch0: strict-upper -25 tri; ch1: identity
    outQ = dram("outQ", (T, 1024), BF, out=True)
    qkA_r = [DqkA[g].rearrange("c p t -> p c t") for g in range(4)]
    qkB_r = [DqkB[g].rearrange("c p t -> p c t") for g in range(4)]
    vm_r = [Dvm[g].rearrange("c p d -> p c d") for g in range(4)]
    outr = outQ.rearrange("(c p) d -> p c d", p=P)

    with ExitStack() as ctx:
        tc = ctx.enter_context(tile.TileContext(nc))
        pers = ctx.enter_context(tc.tile_pool(name="pers", bufs=1))

        qkAp = ctx.enter_context(tc.tile_pool(name="qkA", bufs=2))
        qkBp = ctx.enter_context(tc.tile_pool(name="qkB", bufs=2))
        vmp = ctx.enter_context(tc.tile_pool(name="vm", bufs=2))
        ptp = ctx.enter_context(tc.tile_pool(name="pt", bufs=5))
        tAp = ctx.enter_context(tc.tile_pool(name="tA", bufs=2))
        outp = ctx.enter_context(tc.tile_pool(name="out", bufs=2))
        scrp = ctx.enter_context(tc.tile_pool(name="scr", bufs=3))
        spsum = ctx.enter_context(tc.tile_pool(name="sp", bufs=2, space="PSUM"))
        ypsum = ctx.enter_context(tc.tile_pool(name="yp", bufs=4, space="PSUM"))

        msk = pers.tile([P, 2, P], BF, name="msk")
        tri = msk[:, 0, :]   # 0/1 mask: tri[k,q] = 1 iff q >= k
        load = {"act": 0.0, "dve": 0.0}
        state = {}

        def balanced_exp(pt, c, q0, sT):
            sz = T - q0
            ca = 0.833 * sz + 185
            cd = 1.0417 * sz + 125
            if load["act"] + ca <= load["dve"] + cd:
                load["act"] += ca
                nc.scalar.activation(pt[:, c, q0:T], sT[:, q0:T], Exp)
            else:
                load["dve"] += cd
                nc.vector.tensor_scalar(
                    out=pt[:, c, q0:T].bitcast(I16), in0=sT[:, q0:T],
                    scalar1=MAGIC_A, scalar2=MAGIC_B, op0=mult, op1=add)

        def balanced_normA(tA, qc, y, rec):
            load["dve"] += 1.0417 * 256 + 125
            nc.vector.tensor_scalar(out=tA[:, qc, :], in0=y[:, 0:256],
                                    scalar1=rec, scalar2=None, op0=mult)

        def balanced_accB(out_sl, y, rec, tA_sl):
            load["dve"] += 1.0417 * 128 + 125
            nc.vector.scalar_tensor_tensor(out=out_sl, in0=y[:, 0:128],
                                           scalar=rec, in1=tA_sl,
                                           op0=mult, op1=add)

        def prefetch(g):
            if g >= 4 or ("qkA", g) in state:
                return
            qkA = qkAp.tile([P, 4, T], FP8, tag="qkA", name="qkA")
            nc.sync.dma_start(out=qkA, in_=qkA_r[g])
            qkB = qkBp.tile([64, 6, T], FP8, tag="qkB", name="qkB")
            nc.sync.dma_start(out=qkB, in_=qkB_r[g])
            vmt = vmp.tile([P, TK, 386], BF, tag="vm", name="vm")
            nc.sync.dma_start(out=vmt, in_=vm_r[g])
            state[("qkA", g)] = qkA
            state[("qkB", g)] = qkB
            state[("vm", g)] = vmt

        # group-0 loads: qkA first (head A0 starts), then the rest
        qkA0 = qkAp.tile([P, 4, T], FP8, tag="qkA", name="qkA0")
        nc.sync.dma_start(out=qkA0, in_=qkA_r[0])
        nc.sync.dma_start(out=msk, in_=Dmsk.rearrange("c p t -> p c t"))
        qkB0 = qkBp.tile([64, 6, T], FP8, tag="qkB", name="qkB0")
        nc.sync.dma_start(out=qkB0, in_=qkB_r[0])
        vm0 = vmp.tile([P, TK, 386], BF, tag="vm", name="vm0")
        nc.sync.dma_start(out=vm0, in_=vm_r[0])
        state[("qkA", 0)] = qkA0
        state[("qkB", 0)] = qkB0
        state[("vm", 0)] = vm0

        class Head:
            """One attention head's emission state (phase1 + phase2)."""

            def __init__(self, g, kind, hh=0):
                self.g, self.kind, self.hh = g, kind, hh
                self.label = f"g{g}.{'A' if kind == 'A' else 'B%d' % hh}"
                self.pt = None
                self.ys = {}
                self.sTs = {}

            def ensure_tiles(self):
                if self.pt is None:
                    self.pt = ptp.tile([P, TK, T], BF, tag="pt", name="pt")
                    self.rec = scrp.tile([P, TK], F32, tag="rec", name="rec")
                    if self.kind == "A":
                        self.tA = tAp.tile([P, TK, 256], BF, tag="tA",
                                           name="tA")
                        state[("tA", self.g)] = self.tA
                    else:
                        if ("o", self.g) not in state:
                            state[("o", self.g)] = outp.tile(
                                [P, TK, 256], BF, tag="outt", name="outt")
                        self.outt = state[("o", self.g)]

            def p1_mms(self, c):
                self.ensure_tiles()
                q0 = P * c
                sT = spsum.tile([P, T], F32, tag="sT", name="sT")
                self.sTs[c] = sT
                pieces = ([(q0, 512), (512, T)] if c < 4 else [(q0, T)])
                if self.kind == "A":
                    qk = state[("qkA", self.g)]
                    lhsT = qk[:, 2:4, q0:q0 + P]
                    rhs = lambda a, b: qk[:, 0:2, a:b]
                else:
                    qk = state[("qkB", self.g)]
                    lhsT = qk[:, 0:2, q0:q0 + P]
                    rhs = lambda a, b: qk[:, 2 + 2 * self.hh:4 + 2 * self.hh,
                                          a:b]
                for (a, b) in pieces:
                    nc.tensor.matmul(sT[:, a:b], lhsT, rhs(a, b),
                                     start=True, stop=True, perf_mode=DR)

            def p1_fin(self, c):
                q0 = P * c
                sT = self.sTs.pop(c)
                balanced_exp(self.pt, c, q0, sT)
                nc.gpsimd.tensor_tensor(self.pt[:, c, q0:q0 + P],
                                        self.pt[:, c, q0:q0 + P], tri, mult)

            def p2_mms(self, qc):
                vm = state[("vm", self.g)]
                y = ypsum.tile([P, 512], F32, tag="y", name="y")
                self.ys[qc] = y
                dcol = 257 if self.kind == "A" else 129
                voff = 0 if self.kind == "A" else 257
                # diag chunk first: its mask dependency is the freshest
                order = ([qc] + list(range(qc))) if qc > 0 else [0]
                for i, c in enumerate(order):
                    nc.tensor.matmul(y[:, 0:dcol],
                                     self.pt[:, c, P * qc:P * qc + P],
                                     vm[:, c, voff:voff + dcol],
                                     start=(i == 0), stop=(i == qc))

            def p2_fin(self, qc):
                y = self.ys.pop(qc)
                dcol = 257 if self.kind == "A" else 129
                nc.vector.reciprocal(self.rec[:, qc:qc + 1],
                                     y[:, dcol - 1:dcol])
                if self.kind == "A":
                    balanced_normA(self.tA, qc, y, self.rec[:, qc:qc + 1])
                else:
                    tA = state[("tA", self.g)]
                    h0 = 128 * self.hh
                    balanced_accB(self.outt[:, qc, h0:h0 + 128], y,
                                  self.rec[:, qc:qc + 1],
                                  tA[:, qc, h0:h0 + 128])
                    if self.hh == 1 and qc % 2 == 1:
                        g = self.g
                        nc.sync.dma_start(
                            out=outr[:, qc - 1:qc + 1, 256 * g:256 * g + 256],
                            in_=self.outt[:, qc - 1:qc + 1, :])

        @contextmanager
        def mark(label):
            a = nc.next_id()
            yield
            PHASE_MARKS.append((a, nc.next_id(), label))

        PHASE_MARKS.clear()

        heads = []
        for g in range(4):
            heads.append(Head(g, "A"))
            heads.append(Head(g, "B", 0))
            heads.append(Head(g, "B", 1))

        # Block-pipelined emission: block i = phase1(head_i) chunk-by-chunk
        # interleaved with phase2(head_{i-1}) qc-by-qc, offset by one slot so
        # each qc's reciprocal sits well behind its y-matmuls in the queues.
        last = len(heads) - 1
        for i in range(len(heads) + 1):
            h1 = heads[i] if i < len(heads) else None
            h2 = heads[i - 1] if i > 0 else None
            lab = f"blk{i}"
            with mark(lab):
                if h1 is not None and h1.kind == "A":
                    prefetch(h1.g + 1)
                for c in range(TK):
                    if h1 is not None:
                        h1.p1_mms(c)
                        h1.p1_fin(c)
                    if h2 is not None:
                        h2.p2_mms(c)
                        if c > 0:
                            h2.p2_fin(c - 1)
                if h2 is not None:
                    h2.p2_fin(TK - 1)

    nc.compile()
    return nc


# ---------------------------------------------------------------------------
# Host side
# ---------------------------------------------------------------------------

def _rope(x, pos):
    """HF-style RoPE applied to x [T, d] at positions pos [T]; f32."""
    d = x.shape[1]
    inv = 1.0 / (10000.0 ** (np.arange(0, d, 2, dtype=np.float32) / d))
    ang = pos.astype(np.float32)[:, None] * inv[None, :]       # [T, d/2]
    ang = np.concatenate([ang, ang], 1)
    c, s = np.cos(ang), np.sin(ang)
    rh = np.concatenate([-x[:, d // 2:], x[:, :d // 2]], 1)
    return x * c + rh * s


def make_core_inputs(q, k, v, pos, weights, s, cfg: KCfg = FULL):
    """q,k,v: [T, 2048] fp32 for one batch; returns per-core input dict."""
    w0, w1, w2, w3 = [np.float32(x) for x in weights]
    fA = np.float32(1.0 / 16.0)
    fB = np.float32(1.0 / math.sqrt(128.0))

    qkA = np.zeros((4, 4, P, T), np.float32)
    qkB = np.zeros((4, 6, 64, T), np.float32)
    vm = np.zeros((4, TK, P, 386), np.float32)
    for g in range(4):
        H = 4 * s + g
        # config A (h=8, d_max=256): e=1024 -> d=128 (w0), e=2048 -> d=256 (w1)
        qmA = w1 * _rope(q[:, 256 * H:256 * H + 256], pos)
        qmA[:, :128] += w0 * _rope(q[:, 128 * H:128 * H + 128], pos)
        kmA = w1 * _rope(k[:, 256 * H:256 * H + 256], pos)
        kmA[:, :128] += w0 * _rope(k[:, 128 * H:128 * H + 128], pos)
        qkA[g, 0] = (fA * qmA[:, :128]).T
        qkA[g, 1] = (fA * qmA[:, 128:]).T
        qkA[g, 2] = kmA[:, :128].T
        qkA[g, 3] = kmA[:, 128:].T
        # config B (h=16, d_max=128): e=1024 -> d=64 (w2), e=2048 -> d=128 (w3)
        kmB = w3 * _rope(k[:, 128 * H:128 * H + 128], pos)
        kmB[:, :64] += w2 * _rope(k[:, 64 * H:64 * H + 64], pos)
        qkB[g, 0] = kmB[:, 0:64].T
        qkB[g, 1] = kmB[:, 64:128].T
        for hh in range(2):
            Hq = 8 * s + 2 * g + hh
            qmB = w3 * _rope(q[:, 128 * Hq:128 * Hq + 128], pos)
            qmB[:, :64] += w2 * _rope(q[:, 64 * Hq:64 * Hq + 64], pos)
            qkB[g, 2 + 2 * hh] = (fB * qmB[:, 0:64]).T
            qkB[g, 3 + 2 * hh] = (fB * qmB[:, 64:128]).T
        # mixed V (+ ones columns for the softmax denominators)
        vA = w1 * v[:, 256 * H:256 * H + 256].copy()
        vA[:, :128] += w0 * v[:, 128 * H:128 * H + 128]
        vB = w3 * v[:, 128 * H:128 * H + 128].copy()
        vB[:, :64] += w2 * v[:, 64 * H:64 * H + 64]
        vm[g, :, :, 0:256] = vA.reshape(TK, P, 256)
        vm[g, :, :, 256] = 1.0
        vm[g, :, :, 257:385] = vB.reshape(TK, P, 128)
        vm[g, :, :, 385] = 1.0

    j, kk = np.mgrid[0:P, 0:P]
    tri = (kk >= j).astype(np.float32)   # tri[k,q] = 1 iff q >= k
    msk = np.stack([tri, np.eye(P, dtype=np.float32)])

    return {"qkA": np.ascontiguousarray(qkA, dtype=NPF8),
            "qkB": np.ascontiguousarray(qkB, dtype=NPF8),
            "vm": np.ascontiguousarray(vm, dtype=NPBF),
            "msk": np.ascontiguousarray(msk, dtype=NPBF)}


_PROGRAM_CACHE = {}
TRACE = False
LAST_RESULT = None


def kernel(q_m, k_m, v_m, weights, attention_mask, position_ids):
    global LAST_RESULT
    from concourse.bass_utils import run_bass_kernel_spmd

    cfg = FULL
    q_m = np.asarray(q_m, np.float32)
    k_m = np.asarray(k_m, np.float32)
    v_m = np.asarray(v_m, np.float32)
    weights = np.asarray(weights, np.float32)
    attention_mask = np.asarray(attention_mask, np.float32)
    position_ids = np.asarray(position_ids)
    B, Tq, H = q_m.shape

    causal = np.where(np.tril(np.ones((Tq, Tq), bool)), 0.0, NEG).astype(np.float32)
    for b in range(B):
        assert np.array_equal(attention_mask[b, 0], causal), "non-causal mask"

    if "nc" not in _PROGRAM_CACHE:
        _PROGRAM_CACHE["nc"] = build_program(cfg)
    nc = _PROGRAM_CACHE["nc"]

    in_maps = []
    for b in range(B):
        for s in range(2):
            in_maps.append(make_core_inputs(
                q_m[b], k_m[b], v_m[b], position_ids[b], weights, s, cfg))
    res = run_bass_kernel_spmd(nc, in_maps, list(range(8)), trace=TRACE)
    LAST_RESULT = res
    out = np.zeros((B, Tq, H), np.float32)
    for b in range(B):
        for s in range(2):
            out[b, :, 1024 * s:1024 * s + 1024] = \
                res.results[2 * b + s]["outQ"].astype(np.float32)
    return out


# revision 53
# speedup vs baseline: 1.0096x; 1.0038x over previous
"""Trainium2 Bass kernel for nn_MixedAttnHeadEmbed (mixed-head-config attention).

v6: host pre-rope/mix; fp8 DoubleRow QK; exp split ACT/DVE; fine-grained
head-pipelined schedule; divide-based normalize.

Math (per batch b): two attention configs share q_m/k_m/v_m [B,T,2048]:
  A: h=8  heads, d_max=256, mixing e in {1024,2048} -> d in {128,256}, w0,w1
  B: h=16 heads, d_max=128, mixing e in {1024,2048} -> d in {64,128},  w2,w3
Sharding: 8 cores = 4 batches x 2 shards; shard s owns A-heads [4s,4s+4) and
B-heads [8s,8s+8) -> output cols [1024s, 1024s+1024), written [T, 1024].

Device design notes:
 - Host precomputes roped+mixed qm/km (fp8 e4m3, q-side pre-scaled by
   1/sqrt(d_max)) and mixed V (+ones column) in bf16. Scores are tiny
   (range ~[-1, 1]) so fp8 QK costs only ~1e-3 extra error.
 - Phase 1 QK^T uses MatmulPerfMode.DoubleRow: both 128-deep d-chunks are
   contracted in ONE matmul at 0.5 cycles/col.
 - exp is the binding cost: a static balancer splits chunks between ACT
   (true exp) and DVE (Schraudolph fast-exp: int16(s*128/ln2 + 16251)
   bit-viewed as bf16).
 - Strict-upper diag-block mask: 0/1 multiply on GPSIMD post-exp.
 - Phase 2 pt-stationary bf16 with ones column; normalize = tensor_scalar
   DIVIDE by the PSUM denominator column (no reciprocal round-trips);
   B-heads accumulate onto tA via scalar_tensor_tensor divide+add.
 - Schedule: 12 heads stream through paired blocks — block i emits
   phase1(head_i) chunk-by-chunk interleaved with phase2(head_{i-1})
   qc-by-qc, so PE/ACT/DVE/GPSIMD all stay fed and dependency chains
   (mm -> exp -> mask -> phase2 -> divide) are a full block long.
"""

import math
from contextlib import ExitStack, contextmanager
from dataclasses import dataclass

import numpy as np
import ml_dtypes

import concourse.bass as bass
import concourse.mybir as mybir
import concourse.tile as tile
from concourse import bacc

F32 = mybir.dt.float32
BF = mybir.dt.bfloat16
I16 = mybir.dt.int16
FP8 = mybir.dt.float8e4
NPBF = ml_dtypes.bfloat16
NPF8 = mybir.dt.np(FP8)
NEG = -1e9
P = 128
T = 1024
TK = T // P

MAGIC_A = 128.0 / math.log(2.0)
MAGIC_B = 16251.0   # 127*128 - 5.5 (centered approx err) + 0.5 (floor->round)


@dataclass(frozen=True)
class KCfg:
    pass


FULL = KCfg()

PHASE_MARKS = []  # (start_id, end_id, label) for trace analysis

mult = mybir.AluOpType.mult
add = mybir.AluOpType.add
div = mybir.AluOpType.divide
Exp = mybir.ActivationFunctionType.Exp
DR = mybir.MatmulPerfMode.DoubleRow


def build_program(cfg: KCfg = FULL):
    nc = bacc.Bacc("TRN2", target_bir_lowering=False)

    def dram(name, shape, dt, out=False):
        return nc.declare_dram_parameter(name, list(shape), dt, isOutput=out)

    # qkA ch: qmA-i0 qmA-i1 kmA-i0 kmA-i1 (d-chunk i packed for DoubleRow)
    DqkA = dram("qkA", (4, 4, P, T), FP8)
    # qkB ch: kmB-h0 kmB-h1 qmB0-h0 qmB0-h1 qmB1-h0 qmB1-h1 (64-row halves)
    DqkB = dram("qkB", (4, 6, 64, T), FP8)
    Dvm = dram("vm", (4, TK, P, 386), BF)  # [0:256] vmA, 256 ones, [257:385] vmB, 385 ones
    Dmsk = dram("msk", (2, P, P), BF)      # ch0: strict-upper -25 tri; ch1: identity
    outQ = dram("outQ", (T, 1024), BF, out=True)
    qkA_r = [DqkA[g].rearrange("c p t -> p c t") for g in range(4)]
    qkB_r = [DqkB[g].rearrange("c p t -> p c t") for g in range(4)]
    vm_r = [Dvm[g].rearrange("c p d -> p c d") for g in range(4)]
    outr = outQ.rearrange("(c p) d -> p c d", p=P)

    with ExitStack() as ctx:
        tc = ctx.enter_context(tile.TileContext(nc))
        pers = ctx.enter_context(tc.tile_pool(name="pers", bufs=1))

        qkAp = ctx.enter_context(tc.tile_pool(name="qkA", bufs=2))
        qkBp = ctx.enter_context(tc.tile_pool(name="qkB", bufs=2))
        vmp = ctx.enter_context(tc.tile_pool(name="vm", bufs=2))
        ptp = ctx.enter_context(tc.tile_pool(name="pt", bufs=5))
        tAp = ctx.enter_context(tc.tile_pool(name="tA", bufs=2))
        outp = ctx.enter_context(tc.tile_pool(name="out", bufs=2))
        scrp = ctx.enter_context(tc.tile_pool(name="scr", bufs=3))
        spsum = ctx.enter_context(tc.tile_pool(name="sp", bufs=2, space="PSUM"))
        ypsum = ctx.enter_context(tc.tile_pool(name="yp", bufs=4, space="PSUM"))

        msk = pers.tile([P, 2, P], BF, name="msk")
        tri = msk[:, 0, :]   # 0/1 mask: tri[k,q] = 1 iff q >= k
        load = {"act": 0.0, "dve": 0.0}
        state = {}

        def balanced_exp(pt, c, q0, sT):
            sz = T - q0
            ca = 0.833 * sz + 185
            cd = 1.0417 * sz + 125
            if load["act"] + ca <= load["dve"] + cd:
                load["act"] += ca
                nc.scalar.activation(pt[:, c, q0:T], sT[:, q0:T], Exp)
            else:
                load["dve"] += cd
                nc.vector.tensor_scalar(
                    out=pt[:, c, q0:T].bitcast(I16), in0=sT[:, q0:T],
                    scalar1=MAGIC_A, scalar2=MAGIC_B, op0=mult, op1=add)

        def balanced_normA(tA, qc, y, rec):
            load["dve"] += 1.0417 * 256 + 125
            nc.vector.tensor_scalar(out=tA[:, qc, :], in0=y[:, 0:256],
                                    scalar1=rec, scalar2=None, op0=mult)

        def balanced_accB(out_sl, y, rec, tA_sl):
            load["dve"] += 1.0417 * 128 + 125
            nc.vector.scalar_tensor_tensor(out=out_sl, in0=y[:, 0:128],
                                           scalar=rec, in1=tA_sl,
                                           op0=mult, op1=add)

        def prefetch(g):
            if g >= 4 or ("qkA", g) in state:
                return
            qkA = qkAp.tile([P, 4, T], FP8, tag="qkA", name="qkA")
            nc.sync.dma_start(out=qkA, in_=qkA_r[g])
            qkB = qkBp.tile([64, 6, T], FP8, tag="qkB", name="qkB")
            nc.sync.dma_start(out=qkB, in_=qkB_r[g])
            vmt = vmp.tile([P, TK, 386], BF, tag="vm", name="vm")
            nc.sync.dma_start(out=vmt, in_=vm_r[g])
            state[("qkA", g)] = qkA
            state[("qkB", g)] = qkB
            state[("vm", g)] = vmt

        # group-0 loads: qkA first (head A0 starts), then the rest
        qkA0 = qkAp.tile([P, 4, T], FP8, tag="qkA", name="qkA0")
        nc.sync.dma_start(out=qkA0, in_=qkA_r[0])
        nc.sync.dma_start(out=msk, in_=Dmsk.rearrange("c p t -> p c t"))
        qkB0 = qkBp.tile([64, 6, T], FP8, tag="qkB", name="qkB0")
        nc.sync.dma_start(out=qkB0, in_=qkB_r[0])
        vm0 = vmp.tile([P, TK, 386], BF, tag="vm", name="vm0")
        nc.sync.dma_start(out=vm0, in_=vm_r[0])
        state[("qkA", 0)] = qkA0
        state[("qkB", 0)] = qkB0
        state[("vm", 0)] = vm0

        class Head:
            """One attention head's emission state (phase1 + phase2)."""

            def __init__(self, g, kind, hh=0):
                self.g, self.kind, self.hh = g, kind, hh
                self.label = f"g{g}.{'A' if kind == 'A' else 'B%d' % hh}"
                self.pt = None
                self.ys = {}
                self.sTs = {}

            def ensure_tiles(self):
                if self.pt is None:
                    self.pt = ptp.tile([P, TK, T], BF, tag="pt", name="pt")
                    self.rec = scrp.tile([P, TK], F32, tag="rec", name="rec")
                    if self.kind == "A":
                        self.tA = tAp.tile([P, TK, 256], BF, tag="tA",
                                           name="tA")
                        state[("tA", self.g)] = self.tA
                    else:
                        if ("o", self.g) not in state:
                            state[("o", self.g)] = outp.tile(
                                [P, TK, 256], BF, tag="outt", name="outt")
                        self.outt = state[("o", self.g)]

            def p1_mms(self, c):
                self.ensure_tiles()
                q0 = P * c
                sT = spsum.tile([P, T], F32, tag="sT", name="sT")
                self.sTs[c] = sT
                pieces = ([(q0, 512), (512, T)] if c < 4 else [(q0, T)])
                if self.kind == "A":
                    qk = state[("qkA", self.g)]
                    lhsT = qk[:, 2:4, q0:q0 + P]
                    rhs = lambda a, b: qk[:, 0:2, a:b]
                else:
                    qk = state[("qkB", self.g)]
                    lhsT = qk[:, 0:2, q0:q0 + P]
                    rhs = lambda a, b: qk[:, 2 + 2 * self.hh:4 + 2 * self.hh,
                                          a:b]
                for (a, b) in pieces:
                    nc.tensor.matmul(sT[:, a:b], lhsT, rhs(a, b),
                                     start=True, stop=True, perf_mode=DR)

            def p1_fin(self, c):
                q0 = P * c
                sT = self.sTs.pop(c)
                balanced_exp(self.pt, c, q0, sT)
                nc.gpsimd.tensor_tensor(self.pt[:, c, q0:q0 + P],
                                        self.pt[:, c, q0:q0 + P], tri, mult)

            def p2_mms(self, qc):
                vm = state[("vm", self.g)]
                y = ypsum.tile([P, 512], F32, tag="y", name="y")
                self.ys[qc] = y
                dcol = 257 if self.kind == "A" else 129
                voff = 0 if self.kind == "A" else 257
                # diag chunk first: its mask dependency is the freshest
                order = ([qc] + list(range(qc))) if qc > 0 else [0]
                for i, c in enumerate(order):
                    nc.tensor.matmul(y[:, 0:dcol],
                                     self.pt[:, c, P * qc:P * qc + P],
                                     vm[:, c, voff:voff + dcol],
                                     start=(i == 0), stop=(i == qc))

            def p2_fin(self, qc):
                y = self.ys.pop(qc)
                dcol = 257 if self.kind == "A" else 129
                nc.vector.reciprocal(self.rec[:, qc:qc + 1],
                                     y[:, dcol - 1:dcol])
                if self.kind == "A":
                    balanced_normA(self.tA, qc, y, self.rec[:, qc:qc + 1])
                else:
                    tA = state[("tA", self.g)]
                    h0 = 128 * self.hh
                    balanced_accB(self.outt[:, qc, h0:h0 + 128], y,
                                  self.rec[:, qc:qc + 1],
                                  tA[:, qc, h0:h0 + 128])
                    if self.hh == 1 and qc % 2 == 1:
                        g = self.g
                        nc.sync.dma_start(
                            out=outr[:, qc - 1:qc + 1, 256 * g:256 * g + 256],
                            in_=self.outt[:, qc - 1:qc + 1, :])

        @contextmanager
        def mark(label):
            a = nc.next_id()
            yield
            PHASE_MARKS.append((a, nc.next_id(), label))

        PHASE_MARKS.clear()

        heads = []
        for g in range(4):
            heads.append(Head(g, "A"))
            heads.append(Head(g, "B", 0))
            heads.append(Head(g, "B", 1))

        # Block-pipelined emission: block i = phase1(head_i) chunk-by-chunk
        # interleaved with phase2(head_{i-1}) qc-by-qc, offset by one slot so
        # each qc's reciprocal sits well behind its y-matmuls in the queues.
        last = len(heads) - 1
        for i in range(len(heads) + 1):
            h1 = heads[i] if i < len(heads) else None
            h2 = heads[i - 1] if i > 0 else None
            lab = f"blk{i}"
            with mark(lab):
                if h1 is not None and h1.kind == "A":
                    prefetch(h1.g + 1)
                for c in range(TK):
                    if h2 is not None:
                        h2.p2_mms(c)
                    if h1 is not None:
                        h1.p1_mms(c)
                        h1.p1_fin(c)
                    if h2 is not None and c > 0:
                        h2.p2_fin(c - 1)
                if h2 is not None:
                    h2.p2_fin(TK - 1)

    nc.compile()
    return nc


# ---------------------------------------------------------------------------
# Host side
# ---------------------------------------------------------------------------

def _rope(x, pos):
    """HF-style RoPE applied to x [T, d] at positions pos [T]; f32."""
    d = x.shape[1]
    inv = 1.0 / (10000.0 ** (np.arange(0, d, 2, dtype=np.float32) / d))
    ang = pos.astype(np.float32)[:, None] * inv[None, :]       # [T, d/2]
    ang = np.concatenate([ang, ang], 1)
    c, s = np.cos(ang), np.sin(ang)
    rh = np.concatenate([-x[:, d // 2:], x[:, :d // 2]], 1)
    return x * c + rh * s


def make_core_inputs(q, k, v, pos, weights, s, cfg: KCfg = FULL):
    """q,k,v: [T, 2048] fp32 for one batch; returns per-core input dict."""
    w0, w1, w2, w3 = [np.float32(x) for x in weights]
    fA = np.float32(1.0 / 16.0)
    fB = np.float32(1.0 / math.sqrt(128.0))

    qkA = np.zeros((4, 4, P, T), np.float32)
    qkB = np.zeros((4, 6, 64, T), np.float32)
    vm = np.zeros((4, TK, P, 386), np.float32)
    for g in range(4):
        H = 4 * s + g
        # config A (h=8, d_max=256): e=1024 -> d=128 (w0), e=2048 -> d=256 (w1)
        qmA = w1 * _rope(q[:, 256 * H:256 * H + 256], pos)
        qmA[:, :128] += w0 * _rope(q[:, 128 * H:128 * H + 128], pos)
        kmA = w1 * _rope(k[:, 256 * H:256 * H + 256], pos)
        kmA[:, :128] += w0 * _rope(k[:, 128 * H:128 * H + 128], pos)
        qkA[g, 0] = (fA * qmA[:, :128]).T
        qkA[g, 1] = (fA * qmA[:, 128:]).T
        qkA[g, 2] = kmA[:, :128].T
        qkA[g, 3] = kmA[:, 128:].T
        # config B (h=16, d_max=128): e=1024 -> d=64 (w2), e=2048 -> d=128 (w3)
        kmB = w3 * _rope(k[:, 128 * H:128 * H + 128], pos)
        kmB[:, :64] += w2 * _rope(k[:, 64 * H:64 * H + 64], pos)
        qkB[g, 0] = kmB[:, 0:64].T
        qkB[g, 1] = kmB[:, 64:128].T
        for hh in range(2):
            Hq = 8 * s + 2 * g + hh
            qmB = w3 * _rope(q[:, 128 * Hq:128 * Hq + 128], pos)
            qmB[:, :64] += w2 * _rope(q[:, 64 * Hq:64 * Hq + 64], pos)
            qkB[g, 2 + 2 * hh] = (fB * qmB[:, 0:64]).T
            qkB[g, 3 + 2 * hh] = (fB * qmB[:, 64:128]).T
        # mixed V (+ ones columns for the softmax denominators)
        vA = w1 * v[:, 256 * H:256 * H + 256].copy()
        vA[:, :128] += w0 * v[:, 128 * H:128 * H + 128]
        vB = w3 * v[:, 128 * H:128 * H + 128].copy()
        vB[:, :64] += w2 * v[:, 64 * H:64 * H + 64]
        vm[g, :, :, 0:256] = vA.reshape(TK, P, 256)
        vm[g, :, :, 256] = 1.0
        vm[g, :, :, 257:385] = vB.reshape(TK, P, 128)
        vm[g, :, :, 385] = 1.0

    j, kk = np.mgrid[0:P, 0:P]
    tri = (kk >= j).astype(np.float32)   # tri[k,q] = 1 iff q >= k
    msk = np.stack([tri, np.eye(P, dtype=np.float32)])

    return {"qkA": np.ascontiguousarray(qkA, dtype=NPF8),
            "qkB": np.ascontiguousarray(qkB, dtype=NPF8),
            "vm": np.ascontiguousarray(vm, dtype=NPBF),
            "msk": np.ascontiguousarray(msk, dtype=NPBF)}


_PROGRAM_CACHE = {}
TRACE = False
LAST_RESULT = None


def kernel(q_m, k_m, v_m, weights, attention_mask, position_ids):
    global LAST_RESULT
    from concourse.bass_utils import run_bass_kernel_spmd

    cfg = FULL
    q_m = np.asarray(q_m, np.float32)
    k_m = np.asarray(k_m, np.float32)
    v_m = np.asarray(v_m, np.float32)
    weights = np.asarray(weights, np.float32)
    attention_mask = np.asarray(attention_mask, np.float32)
    position_ids = np.asarray(position_ids)
    B, Tq, H = q_m.shape

    causal = np.where(np.tril(np.ones((Tq, Tq), bool)), 0.0, NEG).astype(np.float32)
    for b in range(B):
        assert np.array_equal(attention_mask[b, 0], causal), "non-causal mask"

    if "nc" not in _PROGRAM_CACHE:
        _PROGRAM_CACHE["nc"] = build_program(cfg)
    nc = _PROGRAM_CACHE["nc"]

    in_maps = []
    for b in range(B):
        for s in range(2):
            in_maps.append(make_core_inputs(
                q_m[b], k_m[b], v_m[b], position_ids[b], weights, s, cfg))
    res = run_bass_kernel_spmd(nc, in_maps, list(range(8)), trace=TRACE)
    LAST_RESULT = res
    out = np.zeros((B, Tq, H), np.float32)
    for b in range(B):
        for s in range(2):
            out[b, :, 1024 * s:1024 * s + 1024] = \
                res.results[2 * b + s]["outQ"].astype(np.float32)
    return out
